# revision 9
# baseline (speedup 1.0000x reference)
"""Trainium2 Bass kernel for the MHA problem (B=4, S=1024, D=1024, H=16, dk=dv=64).

Reference semantics (note the unusual softmax over the QUERY axis):
    q = (Q @ W_Q) -> [B,H,S,dk]; k, v likewise
    scores = q k^T / 8            [B,H,Sq,Sk]
    attn = softmax(scores, axis=QUERY)
    out = attn @ v -> heads concat -> @ W_O + Q  -> LayerNorm
    returns (out, attn)

Sharding over 8 cores: core c -> batch b=c//2, head-group g=c%2 (heads
g*8..g*8+7), and output rows [g*512,(g+1)*512) of batch b.

Everything on-device is kept in a TRANSPOSED layout ([feature, token]) so
the query-axis softmax becomes a free-axis softmax:
    X^T via PE transposes; qpT/kpT = W^T X^T slices; vp in natural [k, dv]
    scoresT[k,q] with k on partitions -> softmax along free axis q
    attn_outT[dv,q] = vp^T attnT; pairs exchange query-halves via AllToAll
    y = attn_out @ W_O + residual; LayerNorm over free axis.
attn output tiles are PE-transposed back to [q,k] before DMA out.
"""

import numpy as np

import concourse.bass as bass
import concourse.mybir as mybir
import concourse.tile as tile
from concourse import bacc
from concourse.bass_utils import run_bass_kernel_spmd
from concourse.masks import make_identity

F32 = mybir.dt.float32
P = 128
S = 1024
D = 1024
H = 16
DK = 64
DV = 64
G = 8          # heads per core
B = 4
N_CORES = 8
LN_EPS = 1e-5
AX = mybir.AxisListType.X


def _build_kernel(tc, io):
    from contextlib import ExitStack
    with ExitStack() as ctx:
        _build_kernel_inner(tc, io, ctx)


def _build_kernel_inner(tc, io, ctx):
    nc = tc.nc
    Xq = io["Xq"].ap()
    Xk = io["Xk"].ap()
    Xv = io["Xv"].ap()
    Xres = io["Xres"].ap()
    Wqh, Wkh, Wvh, Woh = io["Wq"].ap(), io["Wk"].ap(), io["Wv"].ap(), io["Wo"].ap()
    gamma, beta = io["gamma"].ap(), io["beta"].ap()
    attn_out = io["attn_out"].ap()
    y_out = io["y_out"].ap()

    const = ctx.enter_context(tc.tile_pool(name="const", bufs=1))
    xin = ctx.enter_context(tc.tile_pool(name="xin", bufs=2))
    # big: X^T (4MB) then aoU (2MB); wpool: Wq/Wk/Wv (2MB) + Wo (4MB)
    big = ctx.enter_context(tc.tile_pool(name="big", bufs=1))
    wpool = ctx.enter_context(tc.tile_pool(name="wpool", bufs=2))
    proj = ctx.enter_context(tc.tile_pool(name="proj", bufs=1))
    atp = ctx.enter_context(tc.tile_pool(name="atp", bufs=3))
    trc = ctx.enter_context(tc.tile_pool(name="trc", bufs=6))
    aos = ctx.enter_context(tc.tile_pool(name="aos", bufs=2))
    yp = ctx.enter_context(tc.tile_pool(name="yp", bufs=2))
    stats = ctx.enter_context(tc.tile_pool(name="stats", bufs=8))
    ps512 = ctx.enter_context(tc.tile_pool(name="ps512", bufs=4, space="PSUM"))
    psao = ctx.enter_context(tc.tile_pool(name="psao", bufs=2, space="PSUM"))
    pstr = ctx.enter_context(tc.tile_pool(name="pstr", bufs=2, space="PSUM"))
    dram = ctx.enter_context(tc.tile_pool(name="dram", bufs=1, space="DRAM"))

    ident = const.tile([P, P], F32)
    make_identity(nc, ident)
    gamma_sb = const.tile([P, D], F32)
    nc.gpsimd.dma_start(
        out=gamma_sb,
        in_=bass.AP(tensor=gamma.tensor, offset=gamma.offset, ap=[[0, P], *gamma.ap]),
    )
    beta_sb = const.tile([P, D], F32)
    nc.gpsimd.dma_start(
        out=beta_sb,
        in_=bass.AP(tensor=beta.tensor, offset=beta.offset, ap=[[0, P], *beta.ap]),
    )
    eps_sb = const.tile([P, 1], F32)
    nc.vector.memset(eps_sb, LN_EPS)

    # ---- phase 1: transpose X, project ----------------------------------
    # qpT/kpT: [128, 4, 1024]; partition+co*128 = local head-col (pair j=co),
    # vp: [128, 8, 512]; partition+so*128 = key row.
    qpT = proj.tile([P, 4, S], F32, name="qpT")
    kpT = proj.tile([P, 4, S], F32, name="kpT")
    vp = proj.tile([P, 8, DV * G], F32, name="vp")

    def load_w(handle, free):
        wt = wpool.tile([P, 8, free], F32, name="w_sb", tag="w")
        nc.sync.dma_start(wt, handle.rearrange("(ko p) c -> p ko c", p=P))
        return wt

    def transpose_x(x_ap):
        """X [1024,1024] -> X^T in SBUF as [128, do(8), 1024]."""
        xt = big.tile([P, 8, S], F32, name="xt", tag="big")
        for so in range(8):
            xrow = xin.tile([P, S], F32, name="xrow")
            nc.sync.dma_start(xrow, x_ap[so * P:(so + 1) * P, :])
            for do in range(8):
                tp = pstr.tile([P, P], F32, name="tp_in")
                nc.tensor.transpose(tp, xrow[:, do * P:(do + 1) * P], ident)
                nc.vector.tensor_copy(out=xt[:, do, so * P:(so + 1) * P], in_=tp)
        return xt

    # q/k projections: out[M=128 cols, N=512 q] = W_slice^T @ X^T
    for which, (x_ap, w_h, dst, scale) in enumerate(
        [(Xq, Wqh, qpT, 0.125), (Xk, Wkh, kpT, None)]
    ):
        xt = transpose_x(x_ap)
        w_sb = load_w(w_h, DV * G)
        for co in range(4):
            for qh in range(2):
                ps = ps512.tile([P, 512], F32, name="ps_proj", tag="ps512")
                for ko in range(8):
                    nc.tensor.matmul(
                        ps,
                        lhsT=w_sb[:, ko, co * P:(co + 1) * P],
                        rhs=xt[:, ko, qh * 512:(qh + 1) * 512],
                        start=(ko == 0),
                        stop=(ko == 7),
                    )
                out_slice = dst[:, co, qh * 512:(qh + 1) * 512]
                if scale is not None:
                    nc.vector.tensor_scalar_mul(out_slice, ps, scale)
                else:
                    nc.vector.tensor_copy(out=out_slice, in_=ps)

    # v projection: natural layout vp[k, dv] = X_v @ W_v
    xt = transpose_x(Xv)
    w_sb = load_w(Wvh, DV * G)
    for so in range(8):
        ps = ps512.tile([P, 512], F32, name="ps_projv", tag="ps512")
        for ko in range(8):
            nc.tensor.matmul(
                ps,
                lhsT=xt[:, ko, so * P:(so + 1) * P],
                rhs=w_sb[:, ko, :],
                start=(ko == 0),
                stop=(ko == 7),
            )
        nc.vector.tensor_copy(out=vp[:, so, :], in_=ps)

    wo_sb = load_w(Woh, D)  # [128, 8, 1024], reuses the "w" slots

    # ---- phase 2: attention per head pair -------------------------------
    ag_in = dram.tile([512, S], F32, name="ag_in")
    ag_out = dram.tile([1024, S], F32, name="ag_out")

    for j in range(4):
        ao_ps = [psao.tile([P, 512], F32, name=f"ao_{qh}", tag="ao") for qh in range(2)]
        for ko in range(8):
            for hh in range(2):
                h = 2 * j + hh
                prow = 64 * hh
                sps = []
                for qh in range(2):
                    ps = ps512.tile([P, 512], F32, name="ps_sc", tag="ps512")
                    nc.tensor.matmul(
                        ps,
                        lhsT=kpT[prow:prow + 64, j, ko * P:(ko + 1) * P],
                        rhs=qpT[prow:prow + 64, j, qh * 512:(qh + 1) * 512],
                        start=True,
                        stop=True,
                    )
                    sps.append(ps)
                nm2 = stats.tile([P, 2], F32, name="nm2")
                for qh in range(2):
                    nc.vector.reduce_max(nm2[:, qh:qh + 1], sps[qh], axis=AX, negate=True)
                nm = stats.tile([P, 1], F32, name="nm")
                nc.vector.tensor_reduce(nm, nm2, axis=AX, op=mybir.AluOpType.min)
                at = atp.tile([P, S], F32, name="at")
                den = stats.tile([P, 2], F32, name="den")
                for qh in range(2):
                    nc.scalar.activation(
                        out=at[:, qh * 512:(qh + 1) * 512],
                        in_=sps[qh],
                        func=mybir.ActivationFunctionType.Exp,
                        bias=nm,
                        scale=1.0,
                        accum_out=den[:, qh:qh + 1],
                    )
                rcp = stats.tile([P, 1], F32, name="rcp")
                nc.vector.reduce_sum(rcp, den, axis=AX)
                nc.vector.reciprocal(rcp, rcp)
                nc.vector.tensor_scalar_mul(at, at, rcp)
                for qh in range(2):
                    nc.tensor.matmul(
                        ao_ps[qh][prow:prow + 64, :],
                        lhsT=vp[:, ko, h * DV:(h + 1) * DV],
                        rhs=at[:, qh * 512:(qh + 1) * 512],
                        start=(ko == 0),
                        stop=(ko == 7),
                        tile_position=(0, prow),
                    )
                # transpose attnT [k,q] back to [q,k] and write out
                for so in range(8):
                    tp = pstr.tile([P, P], F32, name="tp_at", tag="tp_in")
                    nc.tensor.transpose(tp, at[:, so * P:(so + 1) * P], ident)
                    tcp = trc.tile([P, P], F32, name="tcp")
                    nc.vector.tensor_copy(out=tcp, in_=tp)
                    nc.sync.dma_start(
                        attn_out[h, so * P:(so + 1) * P, ko * P:(ko + 1) * P], tcp
                    )
        for qh in range(2):
            st = aos.tile([P, 512], F32, name="aostage")
            nc.vector.tensor_copy(out=st, in_=ao_ps[qh])
            nc.sync.dma_start(ag_in[j * P:(j + 1) * P, qh * 512:(qh + 1) * 512], st)

    # ---- phase 3: gather the partner's head-half within the pair --------
    nc.gpsimd.collective_compute(
        "AllGather",
        mybir.AluOpType.bypass,
        replica_groups=[[0, 1], [2, 3], [4, 5], [6, 7]],
        ins=[ag_in[:].opt()],
        outs=[ag_out[:].opt()],
    )

    # ---- phase 4: W_O + residual + LayerNorm ----------------------------
    # this core keeps query columns [q0, q0+512) with q0 = (partition_id & 1)*512
    pid = nc.partition_id()
    q0r = nc.alloc_registers("q0_regs")
    nc.regs_alu(q0r, pid, 1, mybir.AluOpType.bitwise_and)
    nc.regs_alu(q0r, q0r, 512, mybir.AluOpType.mult)
    q0 = nc.snap(q0r, donate=True, min_val=0, max_val=512)

    aoU = big.tile([P, 8, 512], F32, name="aoU", tag="big")
    ag_view = ag_out[:].rearrange("(k p) q -> p k q", p=P)
    nc.sync.dma_start(aoU, ag_view[:, :, bass.ds(q0, 512)])

    for ro in range(4):
        res = xin.tile([P, D], F32, name="res", tag="xrow")
        nc.sync.dma_start(res, Xres[ro * P:(ro + 1) * P, :])
        y = yp.tile([P, D], F32, name="y")
        for nh in range(2):
            ps = ps512.tile([P, 512], F32, name="ps_wo", tag="ps512")
            for ko in range(8):
                nc.tensor.matmul(
                    ps,
                    lhsT=aoU[:, ko, ro * P:(ro + 1) * P],
                    rhs=wo_sb[:, ko, nh * 512:(nh + 1) * 512],
                    start=(ko == 0),
                    stop=(ko == 7),
                )
            nc.vector.tensor_add(
                out=y[:, nh * 512:(nh + 1) * 512],
                in0=ps,
                in1=res[:, nh * 512:(nh + 1) * 512],
            )
        bst = stats.tile([P, 2, nc.vector.BN_STATS_DIM], F32, name="bst")
        mv = stats.tile([P, nc.vector.BN_AGGR_DIM], F32, name="mv")
        yg = y.rearrange("p (n d) -> p n d", d=512)
        for sub in range(2):
            nc.vector.bn_stats(out=bst[:, sub, :], in_=yg[:, sub, :])
        nc.vector.bn_aggr(out=mv, in_=bst)
        rstd = stats.tile([P, 1], F32, name="rstd")
        nc.scalar.activation(
            out=rstd,
            in_=mv[:, 1:2],
            func=mybir.ActivationFunctionType.Sqrt,
            bias=eps_sb,
            scale=1.0,
        )
        nc.vector.reciprocal(rstd, rstd)
        nc.vector.tensor_scalar(
            out=y,
            in0=y,
            scalar1=mv[:, 0:1],
            scalar2=rstd,
            op0=mybir.AluOpType.subtract,
            op1=mybir.AluOpType.mult,
        )
        nc.vector.tensor_mul(out=y, in0=y, in1=gamma_sb)
        nc.vector.tensor_add(out=y, in0=y, in1=beta_sb)
        nc.sync.dma_start(y_out[ro * P:(ro + 1) * P, :], y)


_CACHED = None


def _get_nc():
    global _CACHED
    if _CACHED is None:
        nc = bacc.Bacc(None, target_bir_lowering=False, debug=False, num_devices=N_CORES)
        io = {}
        io["Xq"] = nc.dram_tensor("Xq", [S, D], F32, kind="ExternalInput")
        io["Xk"] = nc.dram_tensor("Xk", [S, D], F32, kind="ExternalInput")
        io["Xv"] = nc.dram_tensor("Xv", [S, D], F32, kind="ExternalInput")
        io["Xres"] = nc.dram_tensor("Xres", [512, D], F32, kind="ExternalInput")
        io["Wq"] = nc.dram_tensor("Wq", [D, 512], F32, kind="ExternalInput")
        io["Wk"] = nc.dram_tensor("Wk", [D, 512], F32, kind="ExternalInput")
        io["Wv"] = nc.dram_tensor("Wv", [D, 512], F32, kind="ExternalInput")
        io["Wo"] = nc.dram_tensor("Wo", [D, D], F32, kind="ExternalInput")
        io["gamma"] = nc.dram_tensor("gamma", [D], F32, kind="ExternalInput")
        io["beta"] = nc.dram_tensor("beta", [D], F32, kind="ExternalInput")
        io["attn_out"] = nc.dram_tensor("attn_out", [G, S, S], F32, kind="ExternalOutput")
        io["y_out"] = nc.dram_tensor("y_out", [512, D], F32, kind="ExternalOutput")
        with tile.TileContext(nc) as tc:
            _build_kernel(tc, io)
        nc.compile()
        _CACHED = nc
    return _CACHED


def kernel(Q, K, V, mask, W_Q, W_K, W_V, W_O, ln_gamma, ln_beta, **run_kwargs):
    Q = np.asarray(Q, np.float32)
    K = np.asarray(K, np.float32)
    V = np.asarray(V, np.float32)
    W_Q = np.asarray(W_Q, np.float32)
    W_K = np.asarray(W_K, np.float32)
    W_V = np.asarray(W_V, np.float32)
    W_O = np.asarray(W_O, np.float32)
    ln_gamma = np.asarray(ln_gamma, np.float32)
    ln_beta = np.asarray(ln_beta, np.float32)
    # mask is all-False for this problem (fill: zeros) -> masking is a no-op.

    nc = _get_nc()
    in_maps = []
    for c in range(N_CORES):
        b, g = c // 2, c % 2
        cs = slice(g * 512, (g + 1) * 512)
        in_maps.append({
            "Xq": np.ascontiguousarray(Q[b]),
            "Xk": np.ascontiguousarray(K[b]),
            "Xv": np.ascontiguousarray(V[b]),
            "Xres": np.ascontiguousarray(Q[b, cs, :]),
            "Wq": np.ascontiguousarray(W_Q[:, cs]),
            "Wk": np.ascontiguousarray(W_K[:, cs]),
            "Wv": np.ascontiguousarray(W_V[:, cs]),
            "Wo": W_O,
            "gamma": ln_gamma,
            "beta": ln_beta,
        })
    res = run_bass_kernel_spmd(nc, in_maps, core_ids=list(range(N_CORES)), **run_kwargs)

    out = np.empty((B, S, D), np.float32)
    attn = np.empty((B, H, S, S), np.float32)
    for c in range(N_CORES):
        b, g = c // 2, c % 2
        attn[b, g * G:(g + 1) * G] = res.results[c]["attn_out"]
        out[b, g * 512:(g + 1) * 512] = res.results[c]["y_out"]
    if run_kwargs:
        return (out, attn), res
    return out, attn
